# revision 15
# baseline (speedup 1.0000x reference)
"""Trainium2 Bass kernel for nn_MultiHeadAttention (B=2, S=2048, D=1024, H=16).

Sharding: 8 cores; core c handles batch b = c//4 and 4 heads h0 = 4*(c%4).
Each core computes its heads' Q/K/V projections, scores^T, softmax, attn@V,
and the row-parallel W_o partial product. Host gathers: per-batch partial sums
(the W_o all-reduce) and transposes per-head attn^T -> attn.

Compute dtype: bf16 operands with fp32 PSUM accumulation (norm rel err ~6e-3).
attn is written to HBM as bf16 [h, k, q] and upcast/transposed on host.
"""

import os
import sys

import numpy as np
import ml_dtypes

for _p in ("/opt/trn_rl_repo",):
    if os.path.isdir(_p) and _p not in sys.path:
        sys.path.insert(0, _p)

import concourse.bass as bass
import concourse.mybir as mybir
from concourse import bacc
from concourse.tile import TileContext
from concourse.bass_utils import run_bass_kernel_spmd

BF16 = mybir.dt.bfloat16
F32 = mybir.dt.float32
AF = mybir.ActivationFunctionType

D, H, HD = 1024, 16, 64
B, S = 2, 2048
P = 128
NCORES = 8
HPC = 4             # heads per core
DOUT = HPC * HD     # 256 projection cols per core
NDT = D // P        # 8 contraction tiles for projections
NST = S // P        # 16 sequence tiles (k tiles)
QC = 512            # q chunk (one PSUM bank of fp32)
NQC = S // QC       # 4 q chunks


def build_program() -> bass.Bass:
    nc = bacc.Bacc(None, target_bir_lowering=False, debug=False)

    xq_d = nc.dram_tensor("xqT", [D, S], BF16, kind="ExternalInput")
    xk_d = nc.dram_tensor("xkT", [D, S], BF16, kind="ExternalInput")
    xv_d = nc.dram_tensor("xvT", [D, S], BF16, kind="ExternalInput")
    wq_d = nc.dram_tensor("wq", [D, DOUT], BF16, kind="ExternalInput")
    wk_d = nc.dram_tensor("wk", [D, DOUT], BF16, kind="ExternalInput")
    wv_d = nc.dram_tensor("wv", [D, DOUT], BF16, kind="ExternalInput")
    wo_d = nc.dram_tensor("wo", [DOUT, D], BF16, kind="ExternalInput")
    bias_d = nc.dram_tensor("bias", [1, 3 * DOUT], BF16, kind="ExternalInput")
    attn_d = nc.dram_tensor("attnT", [HPC, S, S], BF16, kind="ExternalOutput")
    part_d = nc.dram_tensor("partial", [S, D], F32, kind="ExternalOutput")

    ctx_lp = nc.allow_low_precision("bf16 softmax reciprocal broadcast")
    ctx_lp.__enter__()
    with TileContext(nc) as tc:
        with (
            tc.tile_pool(name="const", bufs=1) as cpool,
            tc.tile_pool(name="persist", bufs=1) as qkpool,
            tc.tile_pool(name="small", bufs=4) as spool,
            tc.tile_pool(name="ps", bufs=2, space="PSUM") as pps,
            tc.tile_pool(name="po", bufs=2, space="PSUM") as ppo,
            tc.tile_pool(name="pb", bufs=2, space="PSUM") as ppb,
        ):
            # constants
            ones = cpool.tile([1, QC], BF16, tag="ones")
            nc.vector.memset(ones[:, :], 1.0)
            ones65 = cpool.tile([65, P], BF16, tag="ones65")
            nc.vector.memset(ones65[:, :], 1.0)
            bias_sb = cpool.tile([1, 3 * DOUT], BF16, tag="bias")
            nc.sync.dma_start(out=bias_sb[:, :], in_=bias_d[:, :])

            # persistent activations
            QT = qkpool.tile([P, 2, S], BF16, tag="QT")   # [hd-pair rows, mt, q]
            KT = qkpool.tile([P, 2, S], BF16, tag="KT")
            Vp = [qkpool.tile([P, NST, HD + 1], BF16, tag=f"Vp{h}", name=f"Vp{h}") for h in range(HPC)]
            cc = [qkpool.tile([HD, S], BF16, tag=f"cc{h}", name=f"cc{h}") for h in range(HPC)]
            wo4 = qkpool.tile([HD, HPC, D], BF16, tag="wo4")
            nc.sync.dma_start(
                out=wo4[:, :, :], in_=wo_d.rearrange("(h p) n -> p h n", p=HD)
            )

            # ---------------- phase 1+2: projections ----------------
            with tc.tile_pool(name="x", bufs=1) as xpool:
                wq = xpool.tile([P, NDT, DOUT], BF16, tag="wq")
                nc.sync.dma_start(
                    out=wq[:, :, :], in_=wq_d.rearrange("(kt p) m -> p kt m", p=P)
                )
                wk = xpool.tile([P, NDT, DOUT], BF16, tag="wk")
                nc.sync.dma_start(
                    out=wk[:, :, :], in_=wk_d.rearrange("(kt p) m -> p kt m", p=P)
                )
                xq = xpool.tile([P, NDT, S], BF16, tag="xq")
                xk = xpool.tile([P, NDT, S], BF16, tag="xk")
                for t_sb, t_d in ((xq, xq_d), (xk, xk_d)):
                    for half in range(2):
                        hs = slice(half * NDT // 2, (half + 1) * NDT // 2)
                        nc.sync.dma_start(
                            out=t_sb[:, hs, :],
                            in_=t_d.rearrange("(kt p) s -> p kt s", p=P)[:, hs, :],
                        )

                # QT / KT: out[dout, q] = sum_d w[d, dout] * xT[d, q] (+ bias)
                for dst, w_sb, x_sb, boff in ((QT, wq, xq, 0), (KT, wk, xk, DOUT)):
                    for mt in range(2):
                        for qc in range(NQC):
                            pt = pps.tile([P, QC], F32, tag="s")
                            for kt in range(NDT):
                                nc.tensor.matmul(
                                    pt[:, :],
                                    w_sb[:, kt, mt * P:(mt + 1) * P],
                                    x_sb[:, kt, qc * QC:(qc + 1) * QC],
                                    start=(kt == 0),
                                    stop=False,
                                )
                            nc.tensor.matmul(
                                pt[:, :],
                                bias_sb[0:1, boff + mt * P: boff + (mt + 1) * P],
                                ones[0:1, :],
                                start=False,
                                stop=True,
                            )
                            nc.scalar.copy(
                                out=dst[:, mt, qc * QC:(qc + 1) * QC], in_=pt[:, :]
                            )

                wv = xpool.tile([P, NDT, DOUT], BF16, tag="wv")
                nc.sync.dma_start(
                    out=wv[:, :, :], in_=wv_d.rearrange("(kt p) m -> p kt m", p=P)
                )
                xv = xpool.tile([P, NDT, S], BF16, tag="xv")
                for half in range(2):
                    hs = slice(half * NDT // 2, (half + 1) * NDT // 2)
                    nc.sync.dma_start(
                        out=xv[:, hs, :],
                        in_=xv_d.rearrange("(kt p) s -> p kt s", p=P)[:, hs, :],
                    )

                # V natural: out[s, dout] = sum_d xvT[d, s] * wv[d, dout] (+ bias)
                for h in range(HPC):
                    nc.vector.memset(Vp[h][:, :, HD:HD + 1], 1.0)
                for st in range(NST):
                    pv = pps.tile([P, QC], F32, tag="s")
                    for kt in range(NDT):
                        nc.tensor.matmul(
                            pv[:, 0:DOUT],
                            xv[:, kt, st * P:(st + 1) * P],
                            wv[:, kt, :],
                            start=(kt == 0),
                            stop=False,
                        )
                    nc.tensor.matmul(
                        pv[:, 0:DOUT],
                        ones[0:1, 0:P],
                        bias_sb[0:1, 2 * DOUT:3 * DOUT],
                        start=False,
                        stop=True,
                    )
                    for h in range(HPC):
                        eng = nc.scalar if h < 2 else nc.vector
                        if eng is nc.scalar:
                            nc.scalar.copy(
                                out=Vp[h][:, st, 0:HD],
                                in_=pv[:, h * HD:(h + 1) * HD],
                            )
                        else:
                            nc.vector.tensor_copy(
                                out=Vp[h][:, st, 0:HD],
                                in_=pv[:, h * HD:(h + 1) * HD],
                            )

            # ---------------- phase 3: attention (software-pipelined) ----------------
            # Chunk (qc, hp) covers heads {2hp, 2hp+1} x q-slice qc. While chunk
            # i's scores stream through PE->ACT(exp), chunk i-1's attn@V matmuls
            # are interleaved into the PE queue so PE never idles on ACT.
            with (
                tc.tile_pool(name="E", bufs=2) as epool,
                tc.tile_pool(name="A", bufs=3) as apool,
            ):
                chunks = [(qc, hp) for qc in range(NQC) for hp in range(2)]

                def emit_attnv(state, ktpair):
                    """attn@V matmuls for ktpair {2k,2k+1} of a previous chunk."""
                    _, _, ech_p, po_p = state
                    for h01 in range(2):
                        h = state[1] * 2 + h01
                        for kt in (2 * ktpair, 2 * ktpair + 1):
                            nc.tensor.matmul(
                                po_p[h01][:, :],
                                Vp[h][:, kt, :],
                                ech_p[h01][:, kt, :],
                                start=(kt == 0),
                                stop=(kt == NST - 1),
                            )

                def emit_post(state):
                    """softmax normalize + attn DMA + scaled out for a chunk."""
                    qc_p, hp_p, ech_p, po_p = state
                    qs_p = slice(qc_p * QC, (qc_p + 1) * QC)
                    s16s = []
                    for h01 in range(2):
                        s16 = spool.tile([HD + 1, QC], BF16, tag="r16")
                        nc.vector.tensor_copy(
                            out=s16[HD:HD + 1, :], in_=po_p[h01][HD:HD + 1, :]
                        )
                        s16s.append(s16)
                    pbs = []
                    for h01 in range(2):
                        pb = ppb.tile([P, QC], F32, tag="b", name="pb")
                        nc.tensor.matmul(
                            pb[:, :],
                            ones65[HD:HD + 1, :],
                            s16s[h01][HD:HD + 1, :],
                            start=True,
                            stop=True,
                        )
                        pbs.append(pb)
                    bcs = []
                    for h01 in range(2):
                        bc = spool.tile([P, QC], BF16, tag="bc")
                        nc.vector.reciprocal(out=bc[:, :], in_=pbs[h01][:, :])
                        bcs.append(bc)
                    for h01 in range(2):
                        h = hp_p * 2 + h01
                        e = ech_p[h01]
                        po = po_p[h01]
                        bc = bcs[h01]
                        a = apool.tile([P, NST, QC], BF16, tag="A")
                        for k4 in range(NST // 4):
                            nc.vector.tensor_mul(
                                out=a[:, 4 * k4:4 * k4 + 4, :],
                                in0=e[:, 4 * k4:4 * k4 + 4, :],
                                in1=bc[:, None, :].broadcast_to([P, 4, QC]),
                            )
                        nc.sync.dma_start(
                            out=attn_d[h].rearrange("(kt p) q -> p kt q", p=P)[
                                :, :, qs_p
                            ],
                            in_=a[:, :, :],
                        )
                        nc.vector.tensor_mul(
                            out=cc[h][:, qs_p], in0=po[0:HD, :], in1=bc[0:HD, :]
                        )

                def emit_final(qc_p):
                    """W_o row-parallel partial for the 4 s-tiles of q-slice qc_p."""
                    for st in range(qc_p * 4, (qc_p + 1) * 4):
                        for nk in range(2):
                            pf = ppb.tile([P, QC], F32, tag="b", name="pf")
                            for h in range(HPC):
                                nc.tensor.matmul(
                                    pf[:, :],
                                    cc[h][:, st * P:(st + 1) * P],
                                    wo4[:, h, nk * QC:(nk + 1) * QC],
                                    start=(h == 0),
                                    stop=(h == HPC - 1),
                                )
                            fo = spool.tile([P, QC], F32, tag="fo")
                            nc.scalar.copy(out=fo[:, :], in_=pf[:, :])
                            nc.sync.dma_start(
                                out=part_d[
                                    st * P:(st + 1) * P, nk * QC:(nk + 1) * QC
                                ],
                                in_=fo[:, :],
                            )

                prev = None
                for qc, hp in chunks:
                    qs = slice(qc * QC, (qc + 1) * QC)
                    ech = [
                        epool.tile([P, NST, QC], BF16, tag=f"E{h01}", name=f"E{h01}")
                        for h01 in range(2)
                    ]
                    po_cur = [
                        ppo.tile([HD + 1, QC], F32, tag="o", name="po")
                        for _ in range(2)
                    ]
                    for ktpair in range(NST // 2):
                        for h01 in range(2):
                            rows = slice(h01 * HD, (h01 + 1) * HD)
                            ps = pps.tile([P, 2 * QC], F32, tag="s")
                            for j in range(2):
                                kt = 2 * ktpair + j
                                nc.tensor.matmul(
                                    ps[:, j * QC:(j + 1) * QC],
                                    KT[rows, hp, kt * P:(kt + 1) * P],
                                    QT[rows, hp, qs],
                                    start=True,
                                    stop=True,
                                    skip_group_check=True,
                                )
                            nc.scalar.activation(
                                out=ech[h01][:, 2 * ktpair:2 * ktpair + 2, :],
                                in_=ps[:, :],
                                func=AF.Exp,
                            )
                        if prev is not None:
                            emit_attnv(prev, ktpair)
                    if prev is not None:
                        emit_post(prev)
                        if prev[1] == 1:  # both head-pairs of q-slice done
                            emit_final(prev[0])
                    prev = (qc, hp, ech, po_cur)
                # drain the last chunk
                for ktpair in range(NST // 2):
                    emit_attnv(prev, ktpair)
                emit_post(prev)
                emit_final(prev[0])

    ctx_lp.__exit__(None, None, None)
    return nc


_prog_cache: dict = {}


def _get_program() -> bass.Bass:
    if "nc" not in _prog_cache:
        nc = build_program()
        nc.finalize()
        _prog_cache["nc"] = nc
    return _prog_cache["nc"]


def _bf16(x: np.ndarray) -> np.ndarray:
    return np.ascontiguousarray(x).astype(ml_dtypes.bfloat16)


def make_in_maps(query, key_, value, W_q, b_q, W_k, b_k, W_v, b_v, W_o, b_o):
    sc = 1.0 / np.sqrt(np.float32(HD))
    in_maps = []
    for c in range(NCORES):
        b = c // (NCORES // B)
        h0 = (c % (NCORES // B)) * HPC
        cs, ce = h0 * HD, (h0 + HPC) * HD
        bias = np.concatenate([b_q[cs:ce] * sc, b_k[cs:ce], b_v[cs:ce]])[None, :]
        in_maps.append(
            {
                "xqT": _bf16(np.asarray(query[b]).T),
                "xkT": _bf16(np.asarray(key_[b]).T),
                "xvT": _bf16(np.asarray(value[b]).T),
                "wq": _bf16(np.asarray(W_q)[:, cs:ce] * sc),
                "wk": _bf16(np.asarray(W_k)[:, cs:ce]),
                "wv": _bf16(np.asarray(W_v)[:, cs:ce]),
                "wo": _bf16(np.asarray(W_o)[cs:ce, :]),
                "bias": _bf16(np.asarray(bias)),
            }
        )
    return in_maps


def gather_outputs(results, b_o):
    attn = np.empty((B, H, S, S), np.float32)
    final = np.zeros((B, S, D), np.float32)
    for c in range(NCORES):
        b = c // (NCORES // B)
        h0 = (c % (NCORES // B)) * HPC
        at = np.asarray(results[c]["attnT"])  # [HPC, k, q] bf16
        atf = (
            (at.view(np.uint16).astype(np.uint32) << 16).view(np.float32)
        )
        attn[b, h0:h0 + HPC] = atf.transpose(0, 2, 1)
        final[b] += np.asarray(results[c]["partial"])
    final += np.asarray(b_o, dtype=np.float32)[None, None, :]
    return final, attn


def kernel(query, key_, value, W_q, b_q, W_k, b_k, W_v, b_v, W_o, b_o,
           _trace: bool = False):
    query = np.asarray(query, dtype=np.float32)
    key_ = np.asarray(key_, dtype=np.float32)
    value = np.asarray(value, dtype=np.float32)
    W_q = np.asarray(W_q, dtype=np.float32)
    W_k = np.asarray(W_k, dtype=np.float32)
    W_v = np.asarray(W_v, dtype=np.float32)
    W_o = np.asarray(W_o, dtype=np.float32)
    b_q = np.asarray(b_q, dtype=np.float32)
    b_k = np.asarray(b_k, dtype=np.float32)
    b_v = np.asarray(b_v, dtype=np.float32)
    b_o = np.asarray(b_o, dtype=np.float32)

    nc = _get_program()
    in_maps = make_in_maps(
        query, key_, value, W_q, b_q, W_k, b_k, W_v, b_v, W_o, b_o
    )
    res = run_bass_kernel_spmd(nc, in_maps, list(range(NCORES)), trace=_trace)
    _prog_cache["last_results"] = res
    final, attn = gather_outputs(res.results, b_o)
    return final, attn


# revision 16
# speedup vs baseline: 1.0371x; 1.0371x over previous
"""Trainium2 Bass kernel for nn_MultiHeadAttention (B=2, S=2048, D=1024, H=16).

Sharding: 8 cores; core c handles batch b = c//4 and 4 heads h0 = 4*(c%4).
Each core computes its heads' Q/K/V projections, scores^T, softmax, attn@V,
and the row-parallel W_o partial product. Host gathers: per-batch partial sums
(the W_o all-reduce) and transposes per-head attn^T -> attn.

Compute dtype: bf16 operands with fp32 PSUM accumulation (norm rel err ~6e-3).
attn is written to HBM as bf16 [h, k, q] and upcast/transposed on host.
"""

import os
import sys

import numpy as np
import ml_dtypes

for _p in ("/opt/trn_rl_repo",):
    if os.path.isdir(_p) and _p not in sys.path:
        sys.path.insert(0, _p)

import concourse.bass as bass
import concourse.mybir as mybir
from concourse import bacc
from concourse.tile import TileContext
from concourse.bass_utils import run_bass_kernel_spmd

BF16 = mybir.dt.bfloat16
F32 = mybir.dt.float32
AF = mybir.ActivationFunctionType

D, H, HD = 1024, 16, 64
B, S = 2, 2048
P = 128
NCORES = 8
HPC = 4             # heads per core
DOUT = HPC * HD     # 256 projection cols per core
NDT = D // P        # 8 contraction tiles for projections
NST = S // P        # 16 sequence tiles (k tiles)
QC = 512            # q chunk (one PSUM bank of fp32)
NQC = S // QC       # 4 q chunks


def build_program() -> bass.Bass:
    nc = bacc.Bacc(None, target_bir_lowering=False, debug=False)

    xq_d = nc.dram_tensor("xqT", [D, S], BF16, kind="ExternalInput")
    xk_d = nc.dram_tensor("xkT", [D, S], BF16, kind="ExternalInput")
    xv_d = nc.dram_tensor("xvT", [D, S], BF16, kind="ExternalInput")
    wq_d = nc.dram_tensor("wq", [D, DOUT], BF16, kind="ExternalInput")
    wk_d = nc.dram_tensor("wk", [D, DOUT], BF16, kind="ExternalInput")
    wv_d = nc.dram_tensor("wv", [D, DOUT], BF16, kind="ExternalInput")
    wo_d = nc.dram_tensor("wo", [DOUT, D], BF16, kind="ExternalInput")
    bias_d = nc.dram_tensor("bias", [1, 3 * DOUT], BF16, kind="ExternalInput")
    attn_d = nc.dram_tensor("attnT", [HPC, S, S], BF16, kind="ExternalOutput")
    part_d = nc.dram_tensor("partial", [S, D], F32, kind="ExternalOutput")

    ctx_lp = nc.allow_low_precision("bf16 softmax reciprocal broadcast")
    ctx_lp.__enter__()
    with TileContext(nc) as tc:
        with (
            tc.tile_pool(name="const", bufs=1) as cpool,
            tc.tile_pool(name="persist", bufs=1) as qkpool,
            tc.tile_pool(name="small", bufs=4) as spool,
            tc.tile_pool(name="ps", bufs=2, space="PSUM") as pps,
            tc.tile_pool(name="po", bufs=2, space="PSUM") as ppo,
            tc.tile_pool(name="pb", bufs=2, space="PSUM") as ppb,
        ):
            # constants
            ones = cpool.tile([1, QC], BF16, tag="ones")
            nc.vector.memset(ones[:, :], 1.0)
            ones65 = cpool.tile([65, P], BF16, tag="ones65")
            nc.vector.memset(ones65[:, :], 1.0)
            bias_sb = cpool.tile([1, 3 * DOUT], BF16, tag="bias")
            nc.sync.dma_start(out=bias_sb[:, :], in_=bias_d[:, :])

            # persistent activations
            QT = qkpool.tile([P, 2, S], BF16, tag="QT")   # [hd-pair rows, mt, q]
            KT = qkpool.tile([P, 2, S], BF16, tag="KT")
            Vp = [qkpool.tile([P, NST, HD + 1], BF16, tag=f"Vp{h}", name=f"Vp{h}") for h in range(HPC)]
            cc = [qkpool.tile([HD, S], BF16, tag=f"cc{h}", name=f"cc{h}") for h in range(HPC)]
            wo4 = qkpool.tile([HD, HPC, D], BF16, tag="wo4")
            nc.sync.dma_start(
                out=wo4[:, :, :], in_=wo_d.rearrange("(h p) n -> p h n", p=HD)
            )

            # ---------------- phase 1+2: projections ----------------
            with tc.tile_pool(name="x", bufs=1) as xpool:
                wq = xpool.tile([P, NDT, DOUT], BF16, tag="wq")
                nc.sync.dma_start(
                    out=wq[:, :, :], in_=wq_d.rearrange("(kt p) m -> p kt m", p=P)
                )
                wk = xpool.tile([P, NDT, DOUT], BF16, tag="wk")
                nc.sync.dma_start(
                    out=wk[:, :, :], in_=wk_d.rearrange("(kt p) m -> p kt m", p=P)
                )
                xq = xpool.tile([P, NDT, S], BF16, tag="xq")
                xk = xpool.tile([P, NDT, S], BF16, tag="xk")
                for t_sb, t_d in ((xq, xq_d), (xk, xk_d)):
                    for half in range(2):
                        hs = slice(half * NDT // 2, (half + 1) * NDT // 2)
                        nc.sync.dma_start(
                            out=t_sb[:, hs, :],
                            in_=t_d.rearrange("(kt p) s -> p kt s", p=P)[:, hs, :],
                        )

                # QT / KT: out[dout, q] = sum_d w[d, dout] * xT[d, q] (+ bias)
                for dst, w_sb, x_sb, boff in ((QT, wq, xq, 0), (KT, wk, xk, DOUT)):
                    for mt in range(2):
                        for qc in range(NQC):
                            pt = pps.tile([P, QC], F32, tag="s")
                            for kt in range(NDT):
                                nc.tensor.matmul(
                                    pt[:, :],
                                    w_sb[:, kt, mt * P:(mt + 1) * P],
                                    x_sb[:, kt, qc * QC:(qc + 1) * QC],
                                    start=(kt == 0),
                                    stop=False,
                                )
                            nc.tensor.matmul(
                                pt[:, :],
                                bias_sb[0:1, boff + mt * P: boff + (mt + 1) * P],
                                ones[0:1, :],
                                start=False,
                                stop=True,
                            )
                            nc.scalar.copy(
                                out=dst[:, mt, qc * QC:(qc + 1) * QC], in_=pt[:, :]
                            )

                wv = xpool.tile([P, NDT, DOUT], BF16, tag="wv")
                nc.sync.dma_start(
                    out=wv[:, :, :], in_=wv_d.rearrange("(kt p) m -> p kt m", p=P)
                )
                xv = xpool.tile([P, NDT, S], BF16, tag="xv")
                for half in range(2):
                    hs = slice(half * NDT // 2, (half + 1) * NDT // 2)
                    nc.sync.dma_start(
                        out=xv[:, hs, :],
                        in_=xv_d.rearrange("(kt p) s -> p kt s", p=P)[:, hs, :],
                    )

                # V natural: out[s, dout] = sum_d xvT[d, s] * wv[d, dout] (+ bias)
                for h in range(HPC):
                    nc.vector.memset(Vp[h][:, :, HD:HD + 1], 1.0)
                for st in range(NST):
                    pv = pps.tile([P, QC], F32, tag="s")
                    for kt in range(NDT):
                        nc.tensor.matmul(
                            pv[:, 0:DOUT],
                            xv[:, kt, st * P:(st + 1) * P],
                            wv[:, kt, :],
                            start=(kt == 0),
                            stop=False,
                        )
                    nc.tensor.matmul(
                        pv[:, 0:DOUT],
                        ones[0:1, 0:P],
                        bias_sb[0:1, 2 * DOUT:3 * DOUT],
                        start=False,
                        stop=True,
                    )
                    for h in range(HPC):
                        eng = nc.scalar if h < 2 else nc.vector
                        if eng is nc.scalar:
                            nc.scalar.copy(
                                out=Vp[h][:, st, 0:HD],
                                in_=pv[:, h * HD:(h + 1) * HD],
                            )
                        else:
                            nc.vector.tensor_copy(
                                out=Vp[h][:, st, 0:HD],
                                in_=pv[:, h * HD:(h + 1) * HD],
                            )

            # ---------------- phase 3: attention (software-pipelined) ----------------
            # Chunk (qc, hp) covers heads {2hp, 2hp+1} x q-slice qc. While chunk
            # i's scores stream through PE->ACT(exp), chunk i-1's attn@V matmuls
            # are interleaved into the PE queue so PE never idles on ACT.
            with (
                tc.tile_pool(name="E", bufs=2) as epool,
                tc.tile_pool(name="A", bufs=3) as apool,
            ):
                chunks = [(qc, hp) for qc in range(NQC) for hp in range(2)]

                def emit_attnv(state, ktpair):
                    """attn@V matmuls for ktpair {2k,2k+1} of a previous chunk."""
                    _, _, ech_p, po_p = state
                    for h01 in range(2):
                        h = state[1] * 2 + h01
                        for kt in (2 * ktpair, 2 * ktpair + 1):
                            nc.tensor.matmul(
                                po_p[h01][:, :],
                                Vp[h][:, kt, :],
                                ech_p[h01][:, kt, :],
                                start=(kt == 0),
                                stop=(kt == NST - 1),
                            )

                def emit_post(state):
                    """softmax normalize + attn DMA + scaled out for a chunk."""
                    qc_p, hp_p, ech_p, po_p = state
                    qs_p = slice(qc_p * QC, (qc_p + 1) * QC)
                    s16s = []
                    for h01 in range(2):
                        s16 = spool.tile([HD + 1, QC], BF16, tag="r16")
                        nc.vector.tensor_copy(
                            out=s16[HD:HD + 1, :], in_=po_p[h01][HD:HD + 1, :]
                        )
                        s16s.append(s16)
                    pbs = []
                    for h01 in range(2):
                        pb = ppb.tile([P, QC], F32, tag="b", name="pb")
                        nc.tensor.matmul(
                            pb[:, :],
                            ones65[HD:HD + 1, :],
                            s16s[h01][HD:HD + 1, :],
                            start=True,
                            stop=True,
                        )
                        pbs.append(pb)
                    bcs = []
                    for h01 in range(2):
                        bc = spool.tile([P, QC], BF16, tag="bc")
                        nc.vector.reciprocal(out=bc[:, :], in_=pbs[h01][:, :])
                        bcs.append(bc)
                    for h01 in range(2):
                        h = hp_p * 2 + h01
                        e = ech_p[h01]
                        po = po_p[h01]
                        bc = bcs[h01]
                        a = apool.tile([P, NST, QC], BF16, tag="A")
                        for k4 in range(NST // 4):
                            nc.vector.tensor_mul(
                                out=a[:, 4 * k4:4 * k4 + 4, :],
                                in0=e[:, 4 * k4:4 * k4 + 4, :],
                                in1=bc[:, None, :].broadcast_to([P, 4, QC]),
                            )
                        nc.sync.dma_start(
                            out=attn_d[h].rearrange("(kt p) q -> p kt q", p=P)[
                                :, :, qs_p
                            ],
                            in_=a[:, :, :],
                        )
                        nc.vector.tensor_mul(
                            out=cc[h][:, qs_p], in0=po[0:HD, :], in1=bc[0:HD, :]
                        )

                def emit_final(qc_p):
                    """W_o row-parallel partial for the 4 s-tiles of q-slice qc_p."""
                    for st in range(qc_p * 4, (qc_p + 1) * 4):
                        for nk in range(2):
                            pf = ppb.tile([P, QC], F32, tag="b", name="pf")
                            for h in range(HPC):
                                nc.tensor.matmul(
                                    pf[:, :],
                                    cc[h][:, st * P:(st + 1) * P],
                                    wo4[:, h, nk * QC:(nk + 1) * QC],
                                    start=(h == 0),
                                    stop=(h == HPC - 1),
                                )
                            fo = spool.tile([P, QC], F32, tag="fo")
                            nc.scalar.copy(out=fo[:, :], in_=pf[:, :])
                            nc.sync.dma_start(
                                out=part_d[
                                    st * P:(st + 1) * P, nk * QC:(nk + 1) * QC
                                ],
                                in_=fo[:, :],
                            )

                prev = None
                for qc, hp in chunks:
                    qs = slice(qc * QC, (qc + 1) * QC)
                    ech = [
                        epool.tile([P, NST, QC], BF16, tag=f"E{h01}", name=f"E{h01}")
                        for h01 in range(2)
                    ]
                    po_cur = [
                        ppo.tile([HD + 1, QC], F32, tag="o", name="po")
                        for _ in range(2)
                    ]
                    for ktpair in range(NST // 2):
                        for h01 in range(2):
                            rows = slice(h01 * HD, (h01 + 1) * HD)
                            ps = pps.tile([P, 2 * QC], F32, tag="s")
                            for j in range(2):
                                kt = 2 * ktpair + j
                                nc.tensor.matmul(
                                    ps[:, j * QC:(j + 1) * QC],
                                    KT[rows, hp, kt * P:(kt + 1) * P],
                                    QT[rows, hp, qs],
                                    start=True,
                                    stop=True,
                                    skip_group_check=True,
                                )
                            nc.scalar.activation(
                                out=ech[h01][:, 2 * ktpair:2 * ktpair + 2, :],
                                in_=ps[:, :],
                                func=AF.Exp,
                            )
                        if prev is not None:
                            emit_attnv(prev, ktpair)
                    if prev is not None:
                        emit_post(prev)
                        if prev[1] == 1:  # both head-pairs of q-slice done
                            emit_final(prev[0])
                    prev = (qc, hp, ech, po_cur)
                # drain the last chunk
                for ktpair in range(NST // 2):
                    emit_attnv(prev, ktpair)
                emit_post(prev)
                emit_final(prev[0])

    ctx_lp.__exit__(None, None, None)
    return nc


_prog_cache: dict = {}


def _get_program() -> bass.Bass:
    if "nc" not in _prog_cache:
        nc = build_program()
        nc.finalize()
        _prog_cache["nc"] = nc
    return _prog_cache["nc"]


def _bf16(x: np.ndarray) -> np.ndarray:
    return np.ascontiguousarray(x).astype(ml_dtypes.bfloat16)


def make_in_maps(query, key_, value, W_q, b_q, W_k, b_k, W_v, b_v, W_o, b_o):
    sc = 1.0 / np.sqrt(np.float32(HD))
    in_maps = []
    for c in range(NCORES):
        b = c // (NCORES // B)
        h0 = (c % (NCORES // B)) * HPC
        cs, ce = h0 * HD, (h0 + HPC) * HD
        bias = np.concatenate([b_q[cs:ce] * sc, b_k[cs:ce], b_v[cs:ce]])[None, :]
        in_maps.append(
            {
                "xqT": _bf16(np.asarray(query[b]).T),
                "xkT": _bf16(np.asarray(key_[b]).T),
                "xvT": _bf16(np.asarray(value[b]).T),
                "wq": _bf16(np.asarray(W_q)[:, cs:ce] * sc),
                "wk": _bf16(np.asarray(W_k)[:, cs:ce]),
                "wv": _bf16(np.asarray(W_v)[:, cs:ce]),
                "wo": _bf16(np.asarray(W_o)[cs:ce, :]),
                "bias": _bf16(np.asarray(bias)),
            }
        )
    return in_maps


def gather_outputs(results, b_o):
    attn = np.empty((B, H, S, S), np.float32)
    final = np.zeros((B, S, D), np.float32)
    for c in range(NCORES):
        b = c // (NCORES // B)
        h0 = (c % (NCORES // B)) * HPC
        at = np.asarray(results[c]["attnT"])  # [HPC, k, q] bf16
        atf = (
            (at.view(np.uint16).astype(np.uint32) << 16).view(np.float32)
        )
        attn[b, h0:h0 + HPC] = atf.transpose(0, 2, 1)
        final[b] += np.asarray(results[c]["partial"])
    final += np.asarray(b_o, dtype=np.float32)[None, None, :]
    return final, attn


def kernel(query, key_, value, W_q, b_q, W_k, b_k, W_v, b_v, W_o, b_o,
           _trace: bool = False):
    query = np.asarray(query, dtype=np.float32)
    key_ = np.asarray(key_, dtype=np.float32)
    value = np.asarray(value, dtype=np.float32)
    W_q = np.asarray(W_q, dtype=np.float32)
    W_k = np.asarray(W_k, dtype=np.float32)
    W_v = np.asarray(W_v, dtype=np.float32)
    W_o = np.asarray(W_o, dtype=np.float32)
    b_q = np.asarray(b_q, dtype=np.float32)
    b_k = np.asarray(b_k, dtype=np.float32)
    b_v = np.asarray(b_v, dtype=np.float32)
    b_o = np.asarray(b_o, dtype=np.float32)

    nc = _get_program()
    in_maps = make_in_maps(
        query, key_, value, W_q, b_q, W_k, b_k, W_v, b_v, W_o, b_o
    )
    try:
        res = run_bass_kernel_spmd(nc, in_maps, list(range(NCORES)), trace=_trace)
    except Exception:
        # A previous crashed run can leave the NeuronCores unrecoverable;
        # reset through the axon runtime (when present) and retry once.
        try:
            import ctypes

            lib = ctypes.CDLL("/opt/axon/libaxon_pjrt.so")
            lib.axon_reset.restype = ctypes.c_int64
            lib.axon_reset()
        except Exception:
            pass
        res = run_bass_kernel_spmd(nc, in_maps, list(range(NCORES)), trace=_trace)
    _prog_cache["last_results"] = res
    final, attn = gather_outputs(res.results, b_o)
    return final, attn


# revision 17
# speedup vs baseline: 1.0490x; 1.0115x over previous
"""Trainium2 Bass kernel for nn_MultiHeadAttention (B=2, S=2048, D=1024, H=16).

Sharding: 8 cores; core c handles batch b = c//4 and 4 heads h0 = 4*(c%4).
Each core computes its heads' Q/K/V projections, scores^T, softmax, attn@V,
and the row-parallel W_o partial product. Host gathers: per-batch partial sums
(the W_o all-reduce) and transposes per-head attn^T -> attn.

Compute dtype: bf16 operands with fp32 PSUM accumulation (norm rel err ~6e-3).
attn is written to HBM as bf16 [h, k, q] and upcast/transposed on host.
"""

import os
import sys

import numpy as np
import ml_dtypes

for _p in ("/opt/trn_rl_repo",):
    if os.path.isdir(_p) and _p not in sys.path:
        sys.path.insert(0, _p)

import concourse.bass as bass
import concourse.mybir as mybir
from concourse import bacc
from concourse.tile import TileContext
from concourse.bass_utils import run_bass_kernel_spmd

BF16 = mybir.dt.bfloat16
F32 = mybir.dt.float32
AF = mybir.ActivationFunctionType

D, H, HD = 1024, 16, 64
B, S = 2, 2048
P = 128
NCORES = 8
HPC = 4             # heads per core
DOUT = HPC * HD     # 256 projection cols per core
NDT = D // P        # 8 contraction tiles for projections
NST = S // P        # 16 sequence tiles (k tiles)
QC = 512            # q chunk (one PSUM bank of fp32)
NQC = S // QC       # 4 q chunks


def build_program() -> bass.Bass:
    nc = bacc.Bacc(None, target_bir_lowering=False, debug=False)

    xq_d = nc.dram_tensor("xqT", [D, S], BF16, kind="ExternalInput")
    xk_d = nc.dram_tensor("xkT", [D, S], BF16, kind="ExternalInput")
    xv_d = nc.dram_tensor("xvT", [D, S], BF16, kind="ExternalInput")
    wq_d = nc.dram_tensor("wq", [D, DOUT], BF16, kind="ExternalInput")
    wk_d = nc.dram_tensor("wk", [D, DOUT], BF16, kind="ExternalInput")
    wv_d = nc.dram_tensor("wv", [D, DOUT], BF16, kind="ExternalInput")
    wo_d = nc.dram_tensor("wo", [DOUT, D], BF16, kind="ExternalInput")
    bias_d = nc.dram_tensor("bias", [1, 3 * DOUT], BF16, kind="ExternalInput")
    attn_d = nc.dram_tensor("attnT", [HPC, S, S], BF16, kind="ExternalOutput")
    part_d = nc.dram_tensor("partial", [S, D], F32, kind="ExternalOutput")

    ctx_lp = nc.allow_low_precision("bf16 softmax reciprocal broadcast")
    ctx_lp.__enter__()
    with TileContext(nc) as tc:
        with (
            tc.tile_pool(name="const", bufs=1) as cpool,
            tc.tile_pool(name="persist", bufs=1) as qkpool,
            tc.tile_pool(name="small", bufs=4) as spool,
            tc.tile_pool(name="ps", bufs=2, space="PSUM") as pps,
            tc.tile_pool(name="po", bufs=2, space="PSUM") as ppo,
            tc.tile_pool(name="pb", bufs=2, space="PSUM") as ppb,
        ):
            # constants
            ones = cpool.tile([1, QC], BF16, tag="ones")
            nc.vector.memset(ones[:, :], 1.0)
            ones65 = cpool.tile([65, P], BF16, tag="ones65")
            nc.vector.memset(ones65[:, :], 1.0)
            bias_sb = cpool.tile([1, 3 * DOUT], BF16, tag="bias")
            nc.sync.dma_start(out=bias_sb[:, :], in_=bias_d[:, :])

            # persistent activations
            QT = qkpool.tile([P, 2, S], BF16, tag="QT")   # [hd-pair rows, mt, q]
            KT = qkpool.tile([P, 2, S], BF16, tag="KT")
            Vp = [qkpool.tile([P, NST, HD + 1], BF16, tag=f"Vp{h}", name=f"Vp{h}") for h in range(HPC)]
            cc = [qkpool.tile([HD, S], BF16, tag=f"cc{h}", name=f"cc{h}") for h in range(HPC)]
            wo4 = qkpool.tile([HD, HPC, D], BF16, tag="wo4")
            nc.sync.dma_start(
                out=wo4[:, :, :], in_=wo_d.rearrange("(h p) n -> p h n", p=HD)
            )

            # ---------------- phase 1+2: projections ----------------
            with tc.tile_pool(name="x", bufs=1) as xpool:
                wq = xpool.tile([P, NDT, DOUT], BF16, tag="wq")
                nc.sync.dma_start(
                    out=wq[:, :, :], in_=wq_d.rearrange("(kt p) m -> p kt m", p=P)
                )
                wk = xpool.tile([P, NDT, DOUT], BF16, tag="wk")
                nc.sync.dma_start(
                    out=wk[:, :, :], in_=wk_d.rearrange("(kt p) m -> p kt m", p=P)
                )
                xq = xpool.tile([P, NDT, S], BF16, tag="xq")
                xk = xpool.tile([P, NDT, S], BF16, tag="xk")
                for t_sb, t_d in ((xq, xq_d), (xk, xk_d)):
                    for half in range(2):
                        hs = slice(half * NDT // 2, (half + 1) * NDT // 2)
                        nc.sync.dma_start(
                            out=t_sb[:, hs, :],
                            in_=t_d.rearrange("(kt p) s -> p kt s", p=P)[:, hs, :],
                        )

                # QT / KT: out[dout, q] = sum_d w[d, dout] * xT[d, q] (+ bias)
                for dst, w_sb, x_sb, boff in ((QT, wq, xq, 0), (KT, wk, xk, DOUT)):
                    for mt in range(2):
                        for qc in range(NQC):
                            pt = pps.tile([P, QC], F32, tag="s")
                            for kt in range(NDT):
                                nc.tensor.matmul(
                                    pt[:, :],
                                    w_sb[:, kt, mt * P:(mt + 1) * P],
                                    x_sb[:, kt, qc * QC:(qc + 1) * QC],
                                    start=(kt == 0),
                                    stop=False,
                                )
                            nc.tensor.matmul(
                                pt[:, :],
                                bias_sb[0:1, boff + mt * P: boff + (mt + 1) * P],
                                ones[0:1, :],
                                start=False,
                                stop=True,
                            )
                            nc.scalar.copy(
                                out=dst[:, mt, qc * QC:(qc + 1) * QC], in_=pt[:, :]
                            )

                wv = xpool.tile([P, NDT, DOUT], BF16, tag="wv")
                nc.sync.dma_start(
                    out=wv[:, :, :], in_=wv_d.rearrange("(kt p) m -> p kt m", p=P)
                )
                xv = xpool.tile([P, NDT, S], BF16, tag="xv")
                for half in range(2):
                    hs = slice(half * NDT // 2, (half + 1) * NDT // 2)
                    nc.sync.dma_start(
                        out=xv[:, hs, :],
                        in_=xv_d.rearrange("(kt p) s -> p kt s", p=P)[:, hs, :],
                    )

                # V natural: out[s, dout] = sum_d xvT[d, s] * wv[d, dout] (+ bias)
                for h in range(HPC):
                    nc.vector.memset(Vp[h][:, :, HD:HD + 1], 1.0)
                for st in range(NST):
                    pv = pps.tile([P, QC], F32, tag="s")
                    for kt in range(NDT):
                        nc.tensor.matmul(
                            pv[:, 0:DOUT],
                            xv[:, kt, st * P:(st + 1) * P],
                            wv[:, kt, :],
                            start=(kt == 0),
                            stop=False,
                        )
                    nc.tensor.matmul(
                        pv[:, 0:DOUT],
                        ones[0:1, 0:P],
                        bias_sb[0:1, 2 * DOUT:3 * DOUT],
                        start=False,
                        stop=True,
                    )
                    for h in range(HPC):
                        eng = nc.scalar if h < 2 else nc.vector
                        if eng is nc.scalar:
                            nc.scalar.copy(
                                out=Vp[h][:, st, 0:HD],
                                in_=pv[:, h * HD:(h + 1) * HD],
                            )
                        else:
                            nc.vector.tensor_copy(
                                out=Vp[h][:, st, 0:HD],
                                in_=pv[:, h * HD:(h + 1) * HD],
                            )

            # ---------------- phase 3: attention (software-pipelined) ----------------
            # Chunk (qc, hp) covers heads {2hp, 2hp+1} x q-slice qc. While chunk
            # i's scores stream through PE->ACT(exp), chunk i-1's attn@V matmuls
            # are interleaved into the PE queue so PE never idles on ACT.
            with (
                tc.tile_pool(name="E", bufs=2) as epool,
                tc.tile_pool(name="A", bufs=3) as apool,
            ):
                chunks = [(qc, hp) for qc in range(NQC) for hp in range(2)]

                def emit_attnv(state, ktpair):
                    """attn@V matmuls for ktpair {2k,2k+1} of a previous chunk."""
                    _, _, ech_p, po_p = state
                    for h01 in range(2):
                        h = state[1] * 2 + h01
                        for kt in (2 * ktpair, 2 * ktpair + 1):
                            nc.tensor.matmul(
                                po_p[h01][:, :],
                                Vp[h][:, kt, :],
                                ech_p[h01][:, kt, :],
                                start=(kt == 0),
                                stop=(kt == NST - 1),
                            )

                def emit_post(state):
                    """softmax normalize + attn DMA + scaled out for a chunk."""
                    qc_p, hp_p, ech_p, po_p = state
                    qs_p = slice(qc_p * QC, (qc_p + 1) * QC)
                    s16s = []
                    for h01 in range(2):
                        s16 = spool.tile([HD + 1, QC], BF16, tag="r16")
                        nc.vector.tensor_copy(
                            out=s16[HD:HD + 1, :], in_=po_p[h01][HD:HD + 1, :]
                        )
                        s16s.append(s16)
                    pbs = []
                    for h01 in range(2):
                        pb = ppb.tile([P, QC], F32, tag="b", name="pb")
                        nc.tensor.matmul(
                            pb[:, :],
                            ones65[HD:HD + 1, :],
                            s16s[h01][HD:HD + 1, :],
                            start=True,
                            stop=True,
                        )
                        pbs.append(pb)
                    bcs = []
                    for h01 in range(2):
                        bc = spool.tile([P, QC], BF16, tag="bc")
                        nc.vector.reciprocal(out=bc[:, :], in_=pbs[h01][:, :])
                        bcs.append(bc)
                    for h01 in range(2):
                        h = hp_p * 2 + h01
                        e = ech_p[h01]
                        po = po_p[h01]
                        bc = bcs[h01]
                        a = apool.tile([P, NST, QC], BF16, tag="A")
                        for k4 in range(NST // 4):
                            nc.vector.tensor_mul(
                                out=a[:, 4 * k4:4 * k4 + 4, :],
                                in0=e[:, 4 * k4:4 * k4 + 4, :],
                                in1=bc[:, None, :].broadcast_to([P, 4, QC]),
                            )
                        nc.sync.dma_start(
                            out=attn_d[h].rearrange("(kt p) q -> p kt q", p=P)[
                                :, :, qs_p
                            ],
                            in_=a[:, :, :],
                        )
                        nc.vector.tensor_mul(
                            out=cc[h][:, qs_p], in0=po[0:HD, :], in1=bc[0:HD, :]
                        )

                def emit_final(qc_p):
                    """W_o row-parallel partial for the 4 s-tiles of q-slice qc_p."""
                    for st in range(qc_p * 4, (qc_p + 1) * 4):
                        for nk in range(2):
                            pf = ppb.tile([P, QC], F32, tag="b", name="pf")
                            for h in range(HPC):
                                nc.tensor.matmul(
                                    pf[:, :],
                                    cc[h][:, st * P:(st + 1) * P],
                                    wo4[:, h, nk * QC:(nk + 1) * QC],
                                    start=(h == 0),
                                    stop=(h == HPC - 1),
                                )
                            fo = spool.tile([P, QC], F32, tag="fo")
                            nc.scalar.copy(out=fo[:, :], in_=pf[:, :])
                            nc.sync.dma_start(
                                out=part_d[
                                    st * P:(st + 1) * P, nk * QC:(nk + 1) * QC
                                ],
                                in_=fo[:, :],
                            )

                prev = None
                for qc, hp in chunks:
                    qs = slice(qc * QC, (qc + 1) * QC)
                    ech = [
                        epool.tile([P, NST, QC], BF16, tag=f"E{h01}", name=f"E{h01}")
                        for h01 in range(2)
                    ]
                    po_cur = [
                        ppo.tile([HD + 1, QC], F32, tag="o", name="po")
                        for _ in range(2)
                    ]
                    for ktpair in range(NST // 2):
                        pss = [
                            pps.tile([P, 2 * QC], F32, tag="s", name="ps")
                            for _ in range(2)
                        ]
                        # alternate heads: adjacent matmuls hit disjoint PE
                        # row groups (rows 0-63 vs 64-127) and run concurrently
                        for j in range(2):
                            kt = 2 * ktpair + j
                            for h01 in range(2):
                                rows = slice(h01 * HD, (h01 + 1) * HD)
                                nc.tensor.matmul(
                                    pss[h01][:, j * QC:(j + 1) * QC],
                                    KT[rows, hp, kt * P:(kt + 1) * P],
                                    QT[rows, hp, qs],
                                    start=True,
                                    stop=True,
                                    skip_group_check=True,
                                )
                        for h01 in range(2):
                            nc.scalar.activation(
                                out=ech[h01][:, 2 * ktpair:2 * ktpair + 2, :],
                                in_=pss[h01][:, :],
                                func=AF.Exp,
                            )
                        if prev is not None:
                            emit_attnv(prev, ktpair)
                    if prev is not None:
                        emit_post(prev)
                        if prev[1] == 1:  # both head-pairs of q-slice done
                            emit_final(prev[0])
                    prev = (qc, hp, ech, po_cur)
                # drain the last chunk
                for ktpair in range(NST // 2):
                    emit_attnv(prev, ktpair)
                emit_post(prev)
                emit_final(prev[0])

    ctx_lp.__exit__(None, None, None)
    return nc


_prog_cache: dict = {}


def _get_program() -> bass.Bass:
    if "nc" not in _prog_cache:
        nc = build_program()
        nc.finalize()
        _prog_cache["nc"] = nc
    return _prog_cache["nc"]


def _bf16(x: np.ndarray) -> np.ndarray:
    return np.ascontiguousarray(x).astype(ml_dtypes.bfloat16)


def make_in_maps(query, key_, value, W_q, b_q, W_k, b_k, W_v, b_v, W_o, b_o):
    sc = 1.0 / np.sqrt(np.float32(HD))
    in_maps = []
    for c in range(NCORES):
        b = c // (NCORES // B)
        h0 = (c % (NCORES // B)) * HPC
        cs, ce = h0 * HD, (h0 + HPC) * HD
        bias = np.concatenate([b_q[cs:ce] * sc, b_k[cs:ce], b_v[cs:ce]])[None, :]
        in_maps.append(
            {
                "xqT": _bf16(np.asarray(query[b]).T),
                "xkT": _bf16(np.asarray(key_[b]).T),
                "xvT": _bf16(np.asarray(value[b]).T),
                "wq": _bf16(np.asarray(W_q)[:, cs:ce] * sc),
                "wk": _bf16(np.asarray(W_k)[:, cs:ce]),
                "wv": _bf16(np.asarray(W_v)[:, cs:ce]),
                "wo": _bf16(np.asarray(W_o)[cs:ce, :]),
                "bias": _bf16(np.asarray(bias)),
            }
        )
    return in_maps


def gather_outputs(results, b_o):
    attn = np.empty((B, H, S, S), np.float32)
    final = np.zeros((B, S, D), np.float32)
    for c in range(NCORES):
        b = c // (NCORES // B)
        h0 = (c % (NCORES // B)) * HPC
        at = np.asarray(results[c]["attnT"])  # [HPC, k, q] bf16
        atf = (
            (at.view(np.uint16).astype(np.uint32) << 16).view(np.float32)
        )
        attn[b, h0:h0 + HPC] = atf.transpose(0, 2, 1)
        final[b] += np.asarray(results[c]["partial"])
    final += np.asarray(b_o, dtype=np.float32)[None, None, :]
    return final, attn


def kernel(query, key_, value, W_q, b_q, W_k, b_k, W_v, b_v, W_o, b_o,
           _trace: bool = False):
    query = np.asarray(query, dtype=np.float32)
    key_ = np.asarray(key_, dtype=np.float32)
    value = np.asarray(value, dtype=np.float32)
    W_q = np.asarray(W_q, dtype=np.float32)
    W_k = np.asarray(W_k, dtype=np.float32)
    W_v = np.asarray(W_v, dtype=np.float32)
    W_o = np.asarray(W_o, dtype=np.float32)
    b_q = np.asarray(b_q, dtype=np.float32)
    b_k = np.asarray(b_k, dtype=np.float32)
    b_v = np.asarray(b_v, dtype=np.float32)
    b_o = np.asarray(b_o, dtype=np.float32)

    nc = _get_program()
    in_maps = make_in_maps(
        query, key_, value, W_q, b_q, W_k, b_k, W_v, b_v, W_o, b_o
    )
    try:
        res = run_bass_kernel_spmd(nc, in_maps, list(range(NCORES)), trace=_trace)
    except Exception:
        # A previous crashed run can leave the NeuronCores unrecoverable;
        # reset through the axon runtime (when present) and retry once.
        try:
            import ctypes

            lib = ctypes.CDLL("/opt/axon/libaxon_pjrt.so")
            lib.axon_reset.restype = ctypes.c_int64
            lib.axon_reset()
        except Exception:
            pass
        res = run_bass_kernel_spmd(nc, in_maps, list(range(NCORES)), trace=_trace)
    _prog_cache["last_results"] = res
    final, attn = gather_outputs(res.results, b_o)
    return final, attn
